# revision 3
# baseline (speedup 1.0000x reference)
"""GCN 2-layer message-passing encoder on 8 Trainium2 NeuronCores, v2.

Math (matches reference):
    deg  = out-degree(src) + 1 (self loops);  dinv = deg^-1/2
    t1[u] = dinv[u] * x[u] * w1                      (fp16 table)
    out1[v] = dinv[v] * (sum_{e:u->v} t1[u] + t1[v]) + b1
    t2[u] = dinv[u] * relu(out1[u]) * w2             (fp16 table)
    out[v] = dinv[v] * (sum_{e:u->v} t2[u] + t2[v]) + b2

Design vs v1 (5.7ms):
  - dma_gather descriptor generation runs on Q7 core pair (2*q, 2*q+1)
    selected by queue_num: round-robin over 4 SWDGE queues => ~3.5x
    descriptor throughput (measured 2.2ns/idx vs 7.9).
  - fp16 table: half gather bytes, 4x PE matmul rate, 2x DVE one-hot rate.
  - self-loops handled in the flush (elementwise add of own-table row),
    not gathered: fewer idxs, better cell balance.
  - dst buckets processed in groups of 20 held in PSUM, chunk-major inside
    a group, so layer-2 gathers of chunk c only wait on AllGather piece c.
  - table row permutation: node (c,k,j) -> row k*25000 + c*3125 + j, so the
    h AllGather splits into 4 quarter-AGs that pipeline with the sweeps.
  - layer-1 table built locally per core on the ACT engine (no collective).
"""
import numpy as np

import concourse.bacc as bacc
import concourse.bass as bass
import concourse.mybir as mybir
import concourse.tile as tile
from concourse import library_config
from concourse.bass_utils import run_bass_kernel_spmd

P = 128
F32 = mybir.dt.float32
F16 = mybir.dt.float16
I32 = mybir.dt.int32
I16 = mybir.dt.int16

N = 100000
D = 128
NC = 8
SHARD = N // NC            # 12500
NB = (SHARD + P - 1) // P  # 98
QROWS = SHARD // 4         # 3125
CHUNKR = QROWS * NC        # 25000
NCH = 4
GB = 6                     # buckets per PSUM group (PSUM = 8 banks)
NGROUPS = (NB + GB - 1) // GB
TTILES = (N + P - 1) // P  # 782
TROWS = TTILES * P         # 100096
NQ = 4                     # SWDGE queues

LAST_RESULTS = None


def _ceil(a, b):
    return (a + b - 1) // b


# ----------------------------------------------------------------- host side
def preprocess(edges):
    src = np.asarray(edges[:, 0]).astype(np.int64)
    dst = np.asarray(edges[:, 1]).astype(np.int64)

    deg = np.bincount(src, minlength=N).astype(np.float32) + 1.0
    dinv = (deg ** -0.5).astype(np.float32)

    nodes = np.arange(N, dtype=np.int64)
    c_n = nodes // SHARD
    r_n = nodes % SHARD
    trow_of_node = (r_n // QROWS) * CHUNKR + c_n * QROWS + (r_n % QROWS)
    node_of_trow = np.empty(N, np.int64)
    node_of_trow[trow_of_node] = nodes

    tsrc = trow_of_node[src]
    core = dst // SHARD
    brow = dst % SHARD
    bucket = brow // P
    slot = brow % P
    chunk = tsrc // CHUNKR

    # cell sweep order: (group, chunk, bucket)
    cells = []  # (b, ch) in sweep order
    for g in range(NGROUPS):
        for ch in range(NCH):
            for b in range(g * GB, min((g + 1) * GB, NB)):
                cells.append((b, ch))
    n_cells = len(cells)
    cell_rank = np.full((NB, NCH), -1, np.int64)
    for i, (b, ch) in enumerate(cells):
        cell_rank[b, ch] = i

    ckey = cell_rank[bucket, chunk]
    order = np.lexsort((tsrc, ckey, core))
    s_core = core[order]
    s_ckey = ckey[order]
    s_rel = (tsrc - chunk * CHUNKR)[order].astype(np.int16)
    s_slot = slot[order].astype(np.float32)

    cnt = np.bincount(s_core * n_cells + s_ckey,
                      minlength=NC * n_cells).reshape(NC, n_cells)
    maxcnt = cnt.max(axis=0)
    window = np.where(maxcnt > 0, ((maxcnt + 15) // 16) * 16, 0)
    ntl = (window + P - 1) // P
    t0 = np.concatenate([[0], np.cumsum(ntl)])[:-1]
    col0 = np.concatenate([[0], np.cumsum(window // 16)])[:-1]
    T = int(ntl.sum())
    COLS = int((window // 16).sum())

    starts = np.concatenate([[0], np.cumsum(cnt.ravel())])[:-1].reshape(
        NC, n_cells)
    pos = np.arange(s_core.shape[0]) - starts[s_core, s_ckey]

    idx16 = np.full((NC, 16, COLS), -1, np.int16)
    idx16[s_core, pos % 16, col0[s_ckey] + pos // 16] = s_rel
    # a nonempty cell with zero edges on some core still needs >=1 valid idx
    zc, zcell = np.nonzero((cnt == 0) & (window[None, :] > 0))
    idx16[zc, 0, col0[zcell]] = 0
    idx16 = np.tile(idx16, (1, 8, 1))  # [NC, 128, COLS]

    slotv = np.full((NC, P, T), -1.0, np.float32)
    slotv[s_core, pos % P, t0[s_ckey] + pos // P] = s_slot

    # calls: nonempty cells in sweep order
    calls = []  # (b, ch, t0, ntl, window, col0)
    call_rc = []
    for i, (b, ch) in enumerate(cells):
        if window[i] > 0:
            calls.append((b, ch, int(t0[i]), int(ntl[i]), int(window[i]),
                          int(col0[i])))
            call_rc.append(np.maximum(cnt[:, i], 1))
    runcnt = np.stack(call_rc, axis=1).astype(np.int32)  # [NC, n_calls]

    # per bucket: first/last call index in sweep order
    first_call = {}
    last_call = {}
    for ci, (b, ch, *_rest) in enumerate(calls):
        if b not in first_call:
            first_call[b] = ci
        last_call[b] = ci

    dinv_all = np.zeros((P, TTILES), np.float32)
    tr = np.arange(N)
    dinv_all[tr % P, tr // P] = dinv[node_of_trow]

    dinv_shd = np.zeros((NC, P, NB), np.float32)
    vs = np.arange(SHARD)
    for c in range(NC):
        dinv_shd[c, vs % P, vs // P] = dinv[c * SHARD + vs]

    return dict(idx16=idx16, slotv=slotv, runcnt=runcnt, calls=calls,
                first_call=first_call, last_call=last_call,
                dinv_shd=dinv_shd, dinv=dinv,
                node_of_trow=node_of_trow, T=T, COLS=COLS,
                ntl_max=int(ntl.max()), n_calls=len(calls))


# --------------------------------------------------------------- device side
def build_gcn(tc, sched, cfg):
    from contextlib import ExitStack
    ctx = ExitStack()
    nc = tc.nc
    T = sched["T"]
    COLS = sched["COLS"]
    NTLM = sched["ntl_max"]
    n_calls = sched["n_calls"]
    last_pt = SHARD - (NB - 1) * P
    AF = mybir.ActivationFunctionType

    table = nc.dram_tensor("xtab", [TROWS, D], F16, kind="ExternalInput").ap()
    xt_shd = nc.dram_tensor("xt_shd", [NB * P, D], F16,
                            kind="ExternalInput").ap()
    idx_t = nc.dram_tensor("idx", [P, COLS], I16, kind="ExternalInput").ap()
    m16_t = nc.dram_tensor("m16", [P, T + 2 * P], F16,
                           kind="ExternalInput").ap()
    m32w = NB + 4 * D
    m32_t = nc.dram_tensor("m32", [P, m32w], F32, kind="ExternalInput").ap()
    rc_t = nc.dram_tensor("runcnt", [1, n_calls], I32, kind="ExternalInput").ap()
    out_t = nc.dram_tensor("out", [SHARD, D], F32, kind="ExternalOutput").ap()

    dram = ctx.enter_context(tc.tile_pool(name="dram", bufs=1, space="DRAM"))
    htab = [dram.tile([CHUNKR, D], F16, addr_space="Shared", name=f"htab{k}")
            for k in range(NCH)]
    h_shd = dram.tile([SHARD, D], F16, name="h_shd")

    const = ctx.enter_context(tc.tile_pool(name="const", bufs=1))
    idx_sb = const.tile([P, COLS], I16, name="idx_sb")
    m16_sb = const.tile([P, T + 2 * P], F16, name="m16_sb")
    m32_sb = const.tile([P, m32w], F32, name="m32_sb")
    rc_sb = const.tile([1, n_calls], I32, name="rc_sb")
    nc.sync.dma_start(out=rc_sb[:], in_=rc_t[:])
    nc.sync.dma_start(out=m16_sb[:], in_=m16_t[:])
    nc.sync.dma_start(out=m32_sb[:], in_=m32_t[:])
    c_split = sched["calls"][min(48, n_calls - 1)][5]
    if 0 < c_split < COLS:
        nc.sync.dma_start(out=idx_sb[:, :c_split], in_=idx_t[:, :c_split])
        nc.sync.dma_start(out=idx_sb[:, c_split:], in_=idx_t[:, c_split:])
    else:
        nc.sync.dma_start(out=idx_sb[:], in_=idx_t[:])
    slot_sb = m16_sb[:, 0:T]
    iota_sb = m16_sb[:, T:T + P]
    ident_sb = m16_sb[:, T + P:T + 2 * P]
    dinv_shd_sb = m32_sb[:, 0:NB]
    wb = {}
    for i, name in enumerate(("w1b", "b1b", "w2b", "b2b")):
        o = NB + i * D
        wb[name] = m32_sb[:, o:o + D]

    groups = [list(range(NC))]
    nc.gpsimd.load_library(library_config.mlp)

    # ---- edge sweeps
    gp = ctx.enter_context(tc.tile_pool(name="gather", bufs=16))
    op_ = ctx.enter_context(tc.tile_pool(name="onehot", bufs=12))
    pp = ctx.enter_context(tc.tile_pool(name="psum", bufs=8, space="PSUM"))
    fp = ctx.enter_context(tc.tile_pool(name="flush", bufs=10))

    # pre-zero the gather buffers once (padding rows must stay finite;
    # recycled buffers hold old gathered rows, which are finite)
    for i in range(16):
        t = gp.tile([P, NTLM, D], F16, tag="gt")
        nc.vector.memset(t[:], 0.0)
    for i in range(10):
        t2 = fp.tile([P, D], F16, tag="selft", name=f"stz_{i}")
        nc.vector.memset(t2[:], 0.0)

    first_call = sched["first_call"]
    last_call = sched["last_call"]
    # h-AllGather trigger: after flushing bucket tb[k], h rows [0,(k+1)*QROWS)
    # are complete
    trig = {(QROWS * (k + 1) - 1) // P: k for k in range(4)}

    state = {"reg": 0}

    def flush(b, ps, layer):
        pt = P if b < NB - 1 else last_pt
        dv = dinv_shd_sb[:pt, b:b + 1]
        AF = mybir.ActivationFunctionType
        if layer == 1:
            # out1 = ps*dv (+b1); h16 = dinv*relu(out1) (*w2)
            if not cfg["use_b1"]:
                ft = fp.tile([P, D], F32, tag="ft")
                nc.scalar.activation(out=ft[:pt, :], in_=ps[:pt, :],
                                     func=AF.Relu, scale=dv)
                h16 = fp.tile([P, D], F16, tag="h16")
                nc.scalar.activation(out=h16[:pt, :], in_=ft[:pt, :],
                                     func=AF.Copy, scale=dv)
            else:
                ft = fp.tile([P, D], F32, tag="ft")
                nc.vector.tensor_scalar(out=ft[:pt, :], in0=ps[:pt, :],
                                        scalar1=dv, scalar2=None,
                                        op0=mybir.AluOpType.mult)
                nc.vector.tensor_tensor(out=ft[:pt, :], in0=ft[:pt, :],
                                        in1=wb["b1b"][:pt, :],
                                        op=mybir.AluOpType.add)
                h16 = fp.tile([P, D], F16, tag="h16")
                nc.vector.tensor_scalar(out=h16[:pt, :], in0=ft[:pt, :],
                                        scalar1=0.0, scalar2=dv,
                                        op0=mybir.AluOpType.max,
                                        op1=mybir.AluOpType.mult)
            if cfg["use_w2"]:
                nc.vector.tensor_tensor(out=h16[:pt, :], in0=h16[:pt, :],
                                        in1=wb["w2b"][:pt, :],
                                        op=mybir.AluOpType.mult)
            nc.sync.dma_start(out=h_shd[b * P:b * P + pt, :], in_=h16[:pt, :])
            if b in trig:
                k = trig[b]
                nc.gpsimd.collective_compute(
                    "AllGather", mybir.AluOpType.bypass, replica_groups=groups,
                    ins=[h_shd[k * QROWS:(k + 1) * QROWS, :]],
                    outs=[htab[k][:]])
        else:
            fo = fp.tile([P, D], F32, tag="fo")
            if not cfg["use_b2"]:
                nc.scalar.activation(out=fo[:pt, :], in_=ps[:pt, :],
                                     func=AF.Copy, scale=dv)
            else:
                nc.vector.tensor_scalar(out=fo[:pt, :], in0=ps[:pt, :],
                                        scalar1=dv, scalar2=None,
                                        op0=mybir.AluOpType.mult)
            if cfg["use_b2"]:
                nc.vector.tensor_tensor(out=fo[:pt, :], in0=fo[:pt, :],
                                        in1=wb["b2b"][:pt, :],
                                        op=mybir.AluOpType.add)
            nc.sync.dma_start(out=out_t[b * P:b * P + pt, :], in_=fo[:pt, :])

    def sweep(table_of, layer):
        ps_of = {}
        st_of = {}
        for ci, (b, ch, t0, ntl, window, col0) in enumerate(sched["calls"]):
            table_ap, lo = table_of(ch)
            gt = gp.tile([P, NTLM, D], F16, tag="gt")
            reg = nc.gpsimd.alloc_register(f"rc_{layer}_{ci}")
            nc.gpsimd.reg_load(reg, rc_sb[0:1, ci:ci + 1])
            nc.gpsimd.dma_gather(
                out_ap=gt[:, :ntl, :],
                in_ap=table_ap[lo:lo + CHUNKR, :],
                idxs_ap=idx_sb[:, col0:col0 + window // 16],
                num_idxs=window, num_idxs_reg=reg, elem_size=D,
                queue_num=state["reg"] % NQ)
            state["reg"] += 1
            oh = op_.tile([P, NTLM, P], F16, tag="oh")
            nc.vector.tensor_tensor(
                out=oh[:, :ntl, :],
                in0=iota_sb[:, None, :].broadcast_to([P, ntl, P]),
                in1=slot_sb[:, t0:t0 + ntl, None].broadcast_to([P, ntl, P]),
                op=mybir.AluOpType.is_equal)
            if ci == first_call[b]:
                ps_of[b] = pp.tile([P, D], F32, tag="ps", name=f"ps_{layer}_{b}")
                pt = P if b < NB - 1 else last_pt
                st_ = fp.tile([P, D], F16, tag="selft",
                              name=f"st_{layer}_{b}")
                self_src = xt_shd if layer == 1 else h_shd
                nc.sync.dma_start(out=st_[:pt, :],
                                  in_=self_src[b * P:b * P + pt, :])
                st_of[b] = st_
            ps = ps_of[b]
            for j in range(ntl):
                nc.tensor.matmul(out=ps[:], lhsT=oh[:, j, :], rhs=gt[:, j, :],
                                 start=(ci == first_call[b] and j == 0),
                                 stop=False)
            if ci == last_call[b]:
                nc.tensor.matmul(out=ps[:], lhsT=ident_sb[:, :],
                                 rhs=st_of[b][:, :], start=False, stop=True)
                flush(b, ps, layer)
                del ps_of[b]
                del st_of[b]

    sweep(lambda ch: (table[:], ch * CHUNKR), 1)
    sweep(lambda ch: (htab[ch][:], 0), 2)
    ctx.close()


# ---------------------------------------------------------------- entry point
def pack_meta16(sched):
    iota = np.broadcast_to(np.arange(P, dtype=np.float16), (P, P))
    ident = np.eye(P, dtype=np.float16)
    return [np.ascontiguousarray(np.concatenate(
        [sched["slotv"][c].astype(np.float16), iota, ident], axis=1))
        for c in range(NC)]


def pack_meta32(sched, c, w1, b1, w2, b2):
    parts = [sched["dinv_shd"][c],
             np.broadcast_to(w1, (P, D)), np.broadcast_to(b1, (P, D)),
             np.broadcast_to(w2, (P, D)), np.broadcast_to(b2, (P, D))]
    return np.ascontiguousarray(
        np.concatenate(parts, axis=1, dtype=np.float32))


def _run(edges, x, weight1, bias1, weight2, bias2, trace=False):
    global LAST_RESULTS
    x = np.ascontiguousarray(np.asarray(x, np.float32))
    sched = preprocess(np.asarray(edges))

    w1 = np.asarray(weight1, np.float32).reshape(-1)
    b1 = np.asarray(bias1, np.float32).reshape(-1)
    w2 = np.asarray(weight2, np.float32).reshape(-1)
    b2 = np.asarray(bias2, np.float32).reshape(-1)
    cfg = dict(use_b1=not np.all(b1 == 0.0),
               use_w2=not np.all(w2 == 1.0), use_b2=not np.all(b2 == 0.0))

    nc = bacc.Bacc("TRN2", target_bir_lowering=False, debug=False,
                   num_devices=NC, num_swdge_queues=NQ)
    with tile.TileContext(nc) as tc:
        build_gcn(tc, sched, cfg)
    nc.compile()

    t1 = sched["dinv"][:, None] * x * w1[None, :]
    xtab = np.zeros((TROWS, D), np.float16)
    xtab[:N] = t1[sched["node_of_trow"]].astype(np.float16)
    t1_16 = t1.astype(np.float16)
    m16s = pack_meta16(sched)

    in_maps = []
    for c in range(NC):
        in_maps.append(dict(
            xtab=xtab,
            xt_shd=np.concatenate([t1_16[c * SHARD:(c + 1) * SHARD],
                                   np.zeros((NB * P - SHARD, D),
                                            np.float16)]),
            idx=np.ascontiguousarray(sched["idx16"][c]),
            m16=m16s[c],
            m32=pack_meta32(sched, c, w1, b1, w2, b2),
            runcnt=np.ascontiguousarray(sched["runcnt"][c:c + 1]),
        ))

    LAST_RESULTS = run_bass_kernel_spmd(
        nc, in_maps, core_ids=list(range(NC)), trace=trace)
    out = np.concatenate([r["out"] for r in LAST_RESULTS.results], axis=0)
    return out


def kernel(edges, x, weight1, bias1, weight2, bias2):
    import os
    return _run(edges, x, weight1, bias1, weight2, bias2,
                trace=bool(os.environ.get("GCN_TRACE")))


# revision 4
# speedup vs baseline: 1.0003x; 1.0003x over previous
"""GCN 2-layer message-passing encoder on 8 Trainium2 NeuronCores, v2.

Math (matches reference):
    deg  = out-degree(src) + 1 (self loops);  dinv = deg^-1/2
    t1[u] = dinv[u] * x[u] * w1                      (fp16 table)
    out1[v] = dinv[v] * (sum_{e:u->v} t1[u] + t1[v]) + b1
    t2[u] = dinv[u] * relu(out1[u]) * w2             (fp16 table)
    out[v] = dinv[v] * (sum_{e:u->v} t2[u] + t2[v]) + b2

Design vs v1 (5.7ms):
  - dma_gather descriptor generation runs on Q7 core pair (2*q, 2*q+1)
    selected by queue_num: round-robin over 4 SWDGE queues => ~3.5x
    descriptor throughput (measured 2.2ns/idx vs 7.9).
  - fp16 table: half gather bytes, 4x PE matmul rate, 2x DVE one-hot rate.
  - self-loops handled in the flush (elementwise add of own-table row),
    not gathered: fewer idxs, better cell balance.
  - dst buckets processed in groups of 20 held in PSUM, chunk-major inside
    a group, so layer-2 gathers of chunk c only wait on AllGather piece c.
  - table row permutation: node (c,k,j) -> row k*25000 + c*3125 + j, so the
    h AllGather splits into 4 quarter-AGs that pipeline with the sweeps.
  - layer-1 table built locally per core on the ACT engine (no collective).
"""
import numpy as np

import concourse.bacc as bacc
import concourse.bass as bass
import concourse.mybir as mybir
import concourse.tile as tile
from concourse import library_config
from concourse.bass_utils import run_bass_kernel_spmd

P = 128
F32 = mybir.dt.float32
F16 = mybir.dt.float16
I32 = mybir.dt.int32
I16 = mybir.dt.int16

N = 100000
D = 128
NC = 8
SHARD = N // NC            # 12500
NB = (SHARD + P - 1) // P  # 98
QROWS = SHARD // 4         # 3125
CHUNKR = QROWS * NC        # 25000
NCH = 4
GB = 7                     # buckets per PSUM group (PSUM = 8 banks)
NGROUPS = (NB + GB - 1) // GB
TTILES = (N + P - 1) // P  # 782
TROWS = TTILES * P         # 100096
NQ = 4                     # SWDGE queues

LAST_RESULTS = None


def _ceil(a, b):
    return (a + b - 1) // b


# ----------------------------------------------------------------- host side
def preprocess(edges):
    src = np.asarray(edges[:, 0]).astype(np.int64)
    dst = np.asarray(edges[:, 1]).astype(np.int64)

    deg = np.bincount(src, minlength=N).astype(np.float32) + 1.0
    dinv = (deg ** -0.5).astype(np.float32)

    nodes = np.arange(N, dtype=np.int64)
    c_n = nodes // SHARD
    r_n = nodes % SHARD
    trow_of_node = (r_n // QROWS) * CHUNKR + c_n * QROWS + (r_n % QROWS)
    node_of_trow = np.empty(N, np.int64)
    node_of_trow[trow_of_node] = nodes

    tsrc = trow_of_node[src]
    core = dst // SHARD
    brow = dst % SHARD
    bucket = brow // P
    slot = brow % P
    chunk = tsrc // CHUNKR

    # cell sweep order: (group, chunk, bucket)
    cells = []  # (b, ch) in sweep order
    for g in range(NGROUPS):
        for ch in range(NCH):
            for b in range(g * GB, min((g + 1) * GB, NB)):
                cells.append((b, ch))
    n_cells = len(cells)
    cell_rank = np.full((NB, NCH), -1, np.int64)
    for i, (b, ch) in enumerate(cells):
        cell_rank[b, ch] = i

    ckey = cell_rank[bucket, chunk]
    order = np.lexsort((tsrc, ckey, core))
    s_core = core[order]
    s_ckey = ckey[order]
    s_rel = (tsrc - chunk * CHUNKR)[order].astype(np.int16)
    s_slot = slot[order].astype(np.float32)

    cnt = np.bincount(s_core * n_cells + s_ckey,
                      minlength=NC * n_cells).reshape(NC, n_cells)
    maxcnt = cnt.max(axis=0)
    window = np.where(maxcnt > 0, ((maxcnt + 15) // 16) * 16, 0)
    ntl = (window + P - 1) // P
    t0 = np.concatenate([[0], np.cumsum(ntl)])[:-1]
    col0 = np.concatenate([[0], np.cumsum(window // 16)])[:-1]
    T = int(ntl.sum())
    COLS = int((window // 16).sum())

    starts = np.concatenate([[0], np.cumsum(cnt.ravel())])[:-1].reshape(
        NC, n_cells)
    pos = np.arange(s_core.shape[0]) - starts[s_core, s_ckey]

    idx16 = np.full((NC, 16, COLS), -1, np.int16)
    idx16[s_core, pos % 16, col0[s_ckey] + pos // 16] = s_rel
    # a nonempty cell with zero edges on some core still needs >=1 valid idx
    zc, zcell = np.nonzero((cnt == 0) & (window[None, :] > 0))
    idx16[zc, 0, col0[zcell]] = 0
    idx16 = np.tile(idx16, (1, 8, 1))  # [NC, 128, COLS]

    slotv = np.full((NC, P, T), -1.0, np.float32)
    slotv[s_core, pos % P, t0[s_ckey] + pos // P] = s_slot

    # calls: nonempty cells in sweep order
    calls = []  # (b, ch, t0, ntl, window, col0)
    call_rc = []
    for i, (b, ch) in enumerate(cells):
        if window[i] > 0:
            calls.append((b, ch, int(t0[i]), int(ntl[i]), int(window[i]),
                          int(col0[i])))
            call_rc.append(np.maximum(cnt[:, i], 1))
    runcnt = np.stack(call_rc, axis=1).astype(np.int32)  # [NC, n_calls]

    # per bucket: first/last call index in sweep order
    first_call = {}
    last_call = {}
    for ci, (b, ch, *_rest) in enumerate(calls):
        if b not in first_call:
            first_call[b] = ci
        last_call[b] = ci

    dinv_all = np.zeros((P, TTILES), np.float32)
    tr = np.arange(N)
    dinv_all[tr % P, tr // P] = dinv[node_of_trow]

    dinv_shd = np.zeros((NC, P, NB), np.float32)
    vs = np.arange(SHARD)
    for c in range(NC):
        dinv_shd[c, vs % P, vs // P] = dinv[c * SHARD + vs]

    return dict(idx16=idx16, slotv=slotv, runcnt=runcnt, calls=calls,
                first_call=first_call, last_call=last_call,
                dinv_shd=dinv_shd, dinv=dinv,
                node_of_trow=node_of_trow, T=T, COLS=COLS,
                ntl_max=int(ntl.max()), n_calls=len(calls))


# --------------------------------------------------------------- device side
def build_gcn(tc, sched, cfg):
    from contextlib import ExitStack
    ctx = ExitStack()
    nc = tc.nc
    T = sched["T"]
    COLS = sched["COLS"]
    NTLM = sched["ntl_max"]
    n_calls = sched["n_calls"]
    last_pt = SHARD - (NB - 1) * P
    AF = mybir.ActivationFunctionType

    table = nc.dram_tensor("xtab", [TROWS, D], F16, kind="ExternalInput").ap()
    xt_shd = nc.dram_tensor("xt_shd", [NB * P, D], F16,
                            kind="ExternalInput").ap()
    idx_t = nc.dram_tensor("idx", [P, COLS], I16, kind="ExternalInput").ap()
    m16_t = nc.dram_tensor("m16", [P, T + 2 * P], F16,
                           kind="ExternalInput").ap()
    m32w = NB + 4 * D
    m32_t = nc.dram_tensor("m32", [P, m32w], F32, kind="ExternalInput").ap()
    rc_t = nc.dram_tensor("runcnt", [1, n_calls], I32, kind="ExternalInput").ap()
    out_t = nc.dram_tensor("out", [SHARD, D], F32, kind="ExternalOutput").ap()

    dram = ctx.enter_context(tc.tile_pool(name="dram", bufs=1, space="DRAM"))
    htab = [dram.tile([CHUNKR, D], F16, addr_space="Shared", name=f"htab{k}")
            for k in range(NCH)]
    h_shd = dram.tile([SHARD, D], F16, name="h_shd")

    const = ctx.enter_context(tc.tile_pool(name="const", bufs=1))
    idx_sb = const.tile([P, COLS], I16, name="idx_sb")
    m16_sb = const.tile([P, T + 2 * P], F16, name="m16_sb")
    m32_sb = const.tile([P, m32w], F32, name="m32_sb")
    rc_sb = const.tile([1, n_calls], I32, name="rc_sb")
    nc.sync.dma_start(out=rc_sb[:], in_=rc_t[:])
    nc.sync.dma_start(out=m16_sb[:], in_=m16_t[:])
    nc.sync.dma_start(out=m32_sb[:], in_=m32_t[:])
    c_split = sched["calls"][min(48, n_calls - 1)][5]
    if 0 < c_split < COLS:
        nc.sync.dma_start(out=idx_sb[:, :c_split], in_=idx_t[:, :c_split])
        nc.sync.dma_start(out=idx_sb[:, c_split:], in_=idx_t[:, c_split:])
    else:
        nc.sync.dma_start(out=idx_sb[:], in_=idx_t[:])
    slot_sb = m16_sb[:, 0:T]
    iota_sb = m16_sb[:, T:T + P]
    ident_sb = m16_sb[:, T + P:T + 2 * P]
    dinv_shd_sb = m32_sb[:, 0:NB]
    wb = {}
    for i, name in enumerate(("w1b", "b1b", "w2b", "b2b")):
        o = NB + i * D
        wb[name] = m32_sb[:, o:o + D]

    groups = [list(range(NC))]
    nc.gpsimd.load_library(library_config.mlp)

    # ---- edge sweeps
    gp = ctx.enter_context(tc.tile_pool(name="gather", bufs=16))
    op_ = ctx.enter_context(tc.tile_pool(name="onehot", bufs=12))
    pp = ctx.enter_context(tc.tile_pool(name="psum", bufs=8, space="PSUM"))
    fp = ctx.enter_context(tc.tile_pool(name="flush", bufs=10))

    # pre-zero the gather buffers once (padding rows must stay finite;
    # recycled buffers hold old gathered rows, which are finite)
    for i in range(16):
        t = gp.tile([P, NTLM, D], F16, tag="gt")
        nc.vector.memset(t[:], 0.0)
    for i in range(10):
        t2 = fp.tile([P, D], F16, tag="selft", name=f"stz_{i}")
        nc.vector.memset(t2[:], 0.0)

    first_call = sched["first_call"]
    last_call = sched["last_call"]
    # h-AllGather trigger: after flushing bucket tb[k], h rows [0,(k+1)*QROWS)
    # are complete
    trig = {(QROWS * (k + 1) - 1) // P: k for k in range(4)}

    state = {"reg": 0}

    def flush(b, ps, layer):
        pt = P if b < NB - 1 else last_pt
        dv = dinv_shd_sb[:pt, b:b + 1]
        AF = mybir.ActivationFunctionType
        if layer == 1:
            # out1 = ps*dv (+b1); h16 = dinv*relu(out1) (*w2)
            if not cfg["use_b1"]:
                ft = fp.tile([P, D], F32, tag="ft")
                nc.scalar.activation(out=ft[:pt, :], in_=ps[:pt, :],
                                     func=AF.Relu, scale=dv)
                h16 = fp.tile([P, D], F16, tag="h16")
                nc.scalar.activation(out=h16[:pt, :], in_=ft[:pt, :],
                                     func=AF.Copy, scale=dv)
            else:
                ft = fp.tile([P, D], F32, tag="ft")
                nc.vector.tensor_scalar(out=ft[:pt, :], in0=ps[:pt, :],
                                        scalar1=dv, scalar2=None,
                                        op0=mybir.AluOpType.mult)
                nc.vector.tensor_tensor(out=ft[:pt, :], in0=ft[:pt, :],
                                        in1=wb["b1b"][:pt, :],
                                        op=mybir.AluOpType.add)
                h16 = fp.tile([P, D], F16, tag="h16")
                nc.vector.tensor_scalar(out=h16[:pt, :], in0=ft[:pt, :],
                                        scalar1=0.0, scalar2=dv,
                                        op0=mybir.AluOpType.max,
                                        op1=mybir.AluOpType.mult)
            if cfg["use_w2"]:
                nc.vector.tensor_tensor(out=h16[:pt, :], in0=h16[:pt, :],
                                        in1=wb["w2b"][:pt, :],
                                        op=mybir.AluOpType.mult)
            nc.sync.dma_start(out=h_shd[b * P:b * P + pt, :], in_=h16[:pt, :])
            if b in trig:
                k = trig[b]
                nc.gpsimd.collective_compute(
                    "AllGather", mybir.AluOpType.bypass, replica_groups=groups,
                    ins=[h_shd[k * QROWS:(k + 1) * QROWS, :]],
                    outs=[htab[k][:]])
        else:
            fo = fp.tile([P, D], F32, tag="fo")
            if not cfg["use_b2"]:
                nc.scalar.activation(out=fo[:pt, :], in_=ps[:pt, :],
                                     func=AF.Copy, scale=dv)
            else:
                nc.vector.tensor_scalar(out=fo[:pt, :], in0=ps[:pt, :],
                                        scalar1=dv, scalar2=None,
                                        op0=mybir.AluOpType.mult)
            if cfg["use_b2"]:
                nc.vector.tensor_tensor(out=fo[:pt, :], in0=fo[:pt, :],
                                        in1=wb["b2b"][:pt, :],
                                        op=mybir.AluOpType.add)
            nc.sync.dma_start(out=out_t[b * P:b * P + pt, :], in_=fo[:pt, :])

    def sweep(table_of, layer):
        ps_of = {}
        st_of = {}
        for ci, (b, ch, t0, ntl, window, col0) in enumerate(sched["calls"]):
            table_ap, lo = table_of(ch)
            gt = gp.tile([P, NTLM, D], F16, tag="gt")
            reg = nc.gpsimd.alloc_register(f"rc_{layer}_{ci}")
            nc.gpsimd.reg_load(reg, rc_sb[0:1, ci:ci + 1])
            nc.gpsimd.dma_gather(
                out_ap=gt[:, :ntl, :],
                in_ap=table_ap[lo:lo + CHUNKR, :],
                idxs_ap=idx_sb[:, col0:col0 + window // 16],
                num_idxs=window, num_idxs_reg=reg, elem_size=D,
                queue_num=state["reg"] % NQ)
            state["reg"] += 1
            oh = op_.tile([P, NTLM, P], F16, tag="oh")
            nc.vector.tensor_tensor(
                out=oh[:, :ntl, :],
                in0=iota_sb[:, None, :].broadcast_to([P, ntl, P]),
                in1=slot_sb[:, t0:t0 + ntl, None].broadcast_to([P, ntl, P]),
                op=mybir.AluOpType.is_equal)
            if ci == first_call[b]:
                ps_of[b] = pp.tile([P, D], F32, tag="ps", name=f"ps_{layer}_{b}")
                pt = P if b < NB - 1 else last_pt
                st_ = fp.tile([P, D], F16, tag="selft",
                              name=f"st_{layer}_{b}")
                self_src = xt_shd if layer == 1 else h_shd
                nc.sync.dma_start(out=st_[:pt, :],
                                  in_=self_src[b * P:b * P + pt, :])
                st_of[b] = st_
            ps = ps_of[b]
            for j in range(ntl):
                nc.tensor.matmul(out=ps[:], lhsT=oh[:, j, :], rhs=gt[:, j, :],
                                 start=(ci == first_call[b] and j == 0),
                                 stop=False)
            if ci == last_call[b]:
                nc.tensor.matmul(out=ps[:], lhsT=ident_sb[:, :],
                                 rhs=st_of[b][:, :], start=False, stop=True)
                flush(b, ps, layer)
                del ps_of[b]
                del st_of[b]

    sweep(lambda ch: (table[:], ch * CHUNKR), 1)
    sweep(lambda ch: (htab[ch][:], 0), 2)
    ctx.close()


# ---------------------------------------------------------------- entry point
def pack_meta16(sched):
    iota = np.broadcast_to(np.arange(P, dtype=np.float16), (P, P))
    ident = np.eye(P, dtype=np.float16)
    return [np.ascontiguousarray(np.concatenate(
        [sched["slotv"][c].astype(np.float16), iota, ident], axis=1))
        for c in range(NC)]


def pack_meta32(sched, c, w1, b1, w2, b2):
    parts = [sched["dinv_shd"][c],
             np.broadcast_to(w1, (P, D)), np.broadcast_to(b1, (P, D)),
             np.broadcast_to(w2, (P, D)), np.broadcast_to(b2, (P, D))]
    return np.ascontiguousarray(
        np.concatenate(parts, axis=1, dtype=np.float32))


def _run(edges, x, weight1, bias1, weight2, bias2, trace=False):
    global LAST_RESULTS
    x = np.ascontiguousarray(np.asarray(x, np.float32))
    sched = preprocess(np.asarray(edges))

    w1 = np.asarray(weight1, np.float32).reshape(-1)
    b1 = np.asarray(bias1, np.float32).reshape(-1)
    w2 = np.asarray(weight2, np.float32).reshape(-1)
    b2 = np.asarray(bias2, np.float32).reshape(-1)
    cfg = dict(use_b1=not np.all(b1 == 0.0),
               use_w2=not np.all(w2 == 1.0), use_b2=not np.all(b2 == 0.0))

    nc = bacc.Bacc("TRN2", target_bir_lowering=False, debug=False,
                   num_devices=NC, num_swdge_queues=NQ)
    with tile.TileContext(nc) as tc:
        build_gcn(tc, sched, cfg)
    nc.compile()

    t1 = sched["dinv"][:, None] * x * w1[None, :]
    xtab = np.zeros((TROWS, D), np.float16)
    xtab[:N] = t1[sched["node_of_trow"]].astype(np.float16)
    t1_16 = t1.astype(np.float16)
    m16s = pack_meta16(sched)

    in_maps = []
    for c in range(NC):
        in_maps.append(dict(
            xtab=xtab,
            xt_shd=np.concatenate([t1_16[c * SHARD:(c + 1) * SHARD],
                                   np.zeros((NB * P - SHARD, D),
                                            np.float16)]),
            idx=np.ascontiguousarray(sched["idx16"][c]),
            m16=m16s[c],
            m32=pack_meta32(sched, c, w1, b1, w2, b2),
            runcnt=np.ascontiguousarray(sched["runcnt"][c:c + 1]),
        ))

    LAST_RESULTS = run_bass_kernel_spmd(
        nc, in_maps, core_ids=list(range(NC)), trace=trace)
    out = np.concatenate([r["out"] for r in LAST_RESULTS.results], axis=0)
    return out


def kernel(edges, x, weight1, bias1, weight2, bias2):
    import os
    return _run(edges, x, weight1, bias1, weight2, bias2,
                trace=bool(os.environ.get("GCN_TRACE")))
